# revision 4
# baseline (speedup 1.0000x reference)
"""Trainium2 Bass kernel for nn_ClusteringModule (vq_codebook, Student-t soft assignment).

reference math (ALPHA=1):
    sq_dist[b,k] = ||z_b||^2 + ||c_k||^2 - 2 z_b.c_k
    norm = sqrt(sq_dist);  s_tmp = 1/(1+norm);  s = s_tmp / rowsum;  c = argmax_k s

Sharding: data-parallel over batch across 8 NeuronCores (16384 rows each),
centroids replicated. Per-core pipeline, per 512-row block:
  - DMA z block natural [128p, 4c, 256d]
  - PE transposes z -> zT (psum), ACT copies psum->sbuf
  - PE matmuls: rank-1 (ones x c_sq row) + 8 accumulating [128d]x[128b x 128k]
    f32 matmuls producing sq_dist partial (c_sq - 2 z.c) in PSUM
  - ACT: z_sq via Square+accum; norm = Sqrt(psum + z_sq bias)  (sqrt table set)
  - DVE: w = norm+1, s_tmp = recip_approx(w), rowsum, s = s_tmp * 1/rowsum,
    argmax via max/max_index
"""
import os
import sys

sys.path.insert(0, "/opt/trn_rl_repo")

import numpy as np
from contextlib import ExitStack

import concourse.bass as bass
import concourse.mybir as mybir
import concourse.tile as tile
from concourse.bass_utils import run_bass_kernel_spmd
from concourse.masks import make_identity

N_CORES = 8
K = 128
D = 256
RPB = 512  # rows per block
FP = mybir.dt.float32

LAST_EXEC_NS = None
LAST_RESULTS = None


def _split_excess_waits(nc, max_waits=1):
    """This toolchain's walrus only accepts one semaphore wait per
    instruction; hoist extra waits onto NOPs on the same engine."""
    for fn in nc.m.functions:
        for bb in fn.blocks:
            new_insts = []
            for inst in bb.instructions:
                si = inst.sync_info
                if si is not None and si.on_wait and len(si.on_wait) > max_waits:
                    waits = list(si.on_wait)
                    extra, keep = waits[:-max_waits], waits[-max_waits:]
                    for i in range(0, len(extra), max_waits):
                        chunk = extra[i : i + max_waits]
                        nop = mybir.InstNoOp(
                            name=f"{inst.name}-ws{i}",
                            engine=inst.engine,
                            ins=[],
                            outs=[],
                            sync_info=mybir.SyncInfo(on_wait=chunk, on_update=[]),
                        )
                        new_insts.append(nop)
                    si.on_wait = keep
                new_insts.append(inst)
            try:
                bb.instructions[:] = new_insts
            except TypeError:
                bb.instructions = new_insts


def _build(b_core):
    assert b_core % RPB == 0
    nblocks = b_core // RPB
    nc = bass.Bass("TRN2", target_bir_lowering=False)
    z = nc.dram_tensor("z", [b_core, D], FP, kind="ExternalInput")
    cen = nc.dram_tensor("cen", [K, D], FP, kind="ExternalInput")
    s_out = nc.dram_tensor("s", [b_core, K], FP, kind="ExternalOutput")
    c_scr = nc.dram_tensor(
        "c_scr", [128, nblocks * 4], mybir.dt.uint32, kind="ExternalOutput"
    )

    AF = mybir.ActivationFunctionType

    with tile.TileContext(nc) as tc:
        with ExitStack() as ctx:
            consts = ctx.enter_context(tc.tile_pool(name="consts", bufs=1))
            zin = ctx.enter_context(tc.tile_pool(name="zin", bufs=3))
            ps_zt = ctx.enter_context(tc.tile_pool(name="ps_zt", bufs=2, space="PSUM"))
            ps_dist = ctx.enter_context(
                tc.tile_pool(name="ps_dist", bufs=2, space="PSUM")
            )
            ps_small = ctx.enter_context(
                tc.tile_pool(name="ps_small", bufs=1, space="PSUM")
            )
            sb_zt = ctx.enter_context(tc.tile_pool(name="sb_zt", bufs=2))
            sb_mid = ctx.enter_context(tc.tile_pool(name="sb_mid", bufs=2))
            sb_out = ctx.enter_context(tc.tile_pool(name="sb_out", bufs=3))
            stats = ctx.enter_context(tc.tile_pool(name="stats", bufs=3))
            scrapp = ctx.enter_context(tc.tile_pool(name="scrap", bufs=2))

            # ---- constants ----
            ident = consts.tile([128, 128], FP)
            make_identity(nc, ident)

            cnat = consts.tile([128, D], FP)
            nc.sync.dma_start(out=cnat, in_=cen[:, :])

            # cT scaled by -2, per 128-d chunk
            cTs = []
            for d in range(2):
                pst = ps_small.tile([128, 128], FP, tag="pst")
                nc.tensor.transpose(pst, cnat[:, d * 128 : (d + 1) * 128], ident)
                t = consts.tile([128, 128], FP, tag=f"cTs{d}")
                nc.scalar.mul(out=t, in_=pst, mul=-2.0)
                cTs.append(t)

            # c_sq row, replicated 4x -> [1, 512]
            csq_col = consts.tile([128, 1], FP)
            cscrap = consts.tile([128, D], FP)
            nc.scalar.activation(
                out=cscrap, in_=cnat, func=AF.Square, accum_out=csq_col
            )
            pst2 = ps_small.tile([128, 128], FP, tag="pst")
            nc.tensor.transpose(pst2[0:1, :], csq_col, ident)
            csq_row4 = consts.tile([1, 512], FP)
            for i in range(4):
                nc.scalar.copy(
                    out=csq_row4[0:1, i * 128 : (i + 1) * 128], in_=pst2[0:1, :]
                )

            ones_row = consts.tile([1, 128], FP)
            nc.vector.memset(ones_row, 1.0)

            c_acc = consts.tile([128, nblocks * 4], mybir.dt.uint32)

            # ---- main loop ----
            for blk in range(nblocks):
                b0 = blk * RPB
                z_nat = zin.tile([128, 4, D], FP)
                nc.sync.dma_start(
                    out=z_nat,
                    in_=z[b0 : b0 + RPB, :].rearrange("(c p) d -> p c d", p=128),
                )

                # transpose z -> zT (2 d-chunks of [128d, 512b])
                zt_ps0 = ps_zt.tile([128, 512], FP, tag="zt0")
                zt_ps1 = ps_zt.tile([128, 512], FP, tag="zt1")
                for c in range(4):
                    nc.tensor.transpose(
                        zt_ps0[:, c * 128 : (c + 1) * 128], z_nat[:, c, 0:128], ident
                    )
                    nc.tensor.transpose(
                        zt_ps1[:, c * 128 : (c + 1) * 128], z_nat[:, c, 128:256], ident
                    )
                zt0 = sb_zt.tile([128, 512], FP, tag="zt0s")
                zt1 = sb_zt.tile([128, 512], FP, tag="zt1s")
                nc.scalar.copy(out=zt0, in_=zt_ps0)
                nc.scalar.copy(out=zt1, in_=zt_ps1)

                # z_sq per chunk (ACT square + accumulate)
                zsq = stats.tile([128, 4], FP, tag="zsq")
                for c in range(4):
                    scrap = scrapp.tile([128, D], FP, tag="zsq_scrap")
                    nc.scalar.activation(
                        out=scrap,
                        in_=z_nat[:, c, :],
                        func=AF.Square,
                        accum_out=zsq[:, c : c + 1],
                    )

                # dist partial = c_sq - 2 z.c  (PSUM accumulation)
                dist = ps_dist.tile([128, 512], FP, tag="dist")
                nc.tensor.matmul(
                    dist, ones_row, csq_row4, start=True, stop=False,
                    skip_group_check=True,
                )
                for c in range(4):
                    sl = slice(c * 128, (c + 1) * 128)
                    nc.tensor.matmul(
                        dist[:, sl], zt0[:, sl], cTs[0], start=False, stop=False,
                        skip_group_check=True,
                    )
                    nc.tensor.matmul(
                        dist[:, sl], zt1[:, sl], cTs[1], start=False, stop=True,
                        skip_group_check=True,
                    )

                # norm = sqrt(dist + z_sq)
                norm = sb_mid.tile([128, 512], FP, tag="norm")
                for c in range(4):
                    sl = slice(c * 128, (c + 1) * 128)
                    nc.scalar.activation(
                        out=norm[:, sl],
                        in_=dist[:, sl],
                        func=AF.Sqrt,
                        bias=zsq[:, c : c + 1],
                        scale=1.0,
                    )

                # s_tmp = 1/(1+norm)
                w = sb_mid.tile([128, 512], FP, tag="w")
                nc.vector.tensor_scalar_add(out=w, in0=norm, scalar1=1.0)
                stmp = sb_mid.tile([128, 512], FP, tag="stmp")
                nc.vector.reciprocal_approx_fast(out=stmp, in_=w)

                # rowsum and reciprocal
                rows = stats.tile([128, 4], FP, tag="rows")
                nc.vector.reduce_sum(
                    out=rows,
                    in_=stmp.rearrange("p (c k) -> p c k", k=128),
                    axis=mybir.AxisListType.X,
                )
                rr = stats.tile([128, 4], FP, tag="rr")
                nc.vector.reciprocal_approx_fast(out=rr, in_=rows)

                # s = s_tmp * rr ; argmax
                s_sb = sb_out.tile([128, 512], FP, tag="s_sb")
                for c in range(4):
                    sl = slice(c * 128, (c + 1) * 128)
                    nc.vector.tensor_scalar_mul(
                        out=s_sb[:, sl], in0=stmp[:, sl], scalar1=rr[:, c : c + 1]
                    )
                    mx = stats.tile([128, 8], FP, tag="mx")
                    nc.vector.max(out=mx, in_=stmp[:, sl])
                    ix = stats.tile([128, 8], mybir.dt.uint32, tag="ix")
                    nc.vector.max_index(out=ix, in_max=mx, in_values=stmp[:, sl])
                    nc.vector.tensor_copy(
                        out=c_acc[:, blk * 4 + c : blk * 4 + c + 1], in_=ix[:, 0:1]
                    )

                nc.sync.dma_start(
                    out=s_out[b0 : b0 + RPB, :].rearrange("(c p) k -> p c k", p=128),
                    in_=s_sb.rearrange("p (c k) -> p c k", k=128),
                )

            nc.sync.dma_start(out=c_scr[:, :], in_=c_acc)

    mybir.codegen_inst_isa_subclasses(nc)  # fill .instr for custom-DVE ISA insts
    _split_excess_waits(nc)
    return nc, nblocks


_BUILD_CACHE = {}


def _get_built(b_core):
    if b_core not in _BUILD_CACHE:
        _BUILD_CACHE[b_core] = _build(b_core)
    return _BUILD_CACHE[b_core]


def _ensure_ntff_hook():
    """Register the axon NTFF profile hook if the image lacks antenv.axon_hooks."""
    try:
        from antenv import axon_hooks  # noqa: F401
        return
    except ImportError:
        pass
    import types
    import antenv
    from trn_agent_boot.trn_boot import _ntff_profile_via_ctypes

    hook = _ntff_profile_via_ctypes("/opt/axon/libaxon_pjrt.so")
    mod = types.ModuleType("antenv.axon_hooks")
    mod._hook = hook
    mod.get_axon_ntff_profile_hook = lambda: mod._hook
    mod.set_axon_ntff_profile_hook = lambda h: setattr(mod, "_hook", h)
    sys.modules["antenv.axon_hooks"] = mod
    antenv.axon_hooks = mod

    import concourse.bass_utils as BU

    if not getattr(BU, "_upload_patched", False):
        _orig_upload = BU.upload_artifacts

        def _safe_upload(tmpdir):
            try:
                return _orig_upload(tmpdir)
            except Exception:
                return str(tmpdir)

        BU.upload_artifacts = _safe_upload
        BU._upload_patched = True


def kernel(z, centroids):
    global LAST_EXEC_NS, LAST_RESULTS
    z = np.ascontiguousarray(np.asarray(z, dtype=np.float32))
    cen = np.ascontiguousarray(np.asarray(centroids, dtype=np.float32))
    B = z.shape[0]
    assert B % N_CORES == 0
    b_core = B // N_CORES
    nc, nblocks = _get_built(b_core)

    ins = [
        {"z": z[i * b_core : (i + 1) * b_core], "cen": cen} for i in range(N_CORES)
    ]
    trace = bool(os.environ.get("KERNEL_TRACE"))
    if trace:
        _ensure_ntff_hook()
    res = run_bass_kernel_spmd(
        nc, ins, core_ids=list(range(N_CORES)), trace=trace
    )
    LAST_EXEC_NS = res.exec_time_ns
    LAST_RESULTS = res

    s = np.concatenate([res.results[i]["s"] for i in range(N_CORES)], axis=0)
    cs = []
    for i in range(N_CORES):
        c_scr = res.results[i]["c_scr"]  # [128, nblocks*4] uint32
        cc = c_scr.reshape(128, nblocks, 4).transpose(1, 2, 0).reshape(-1)
        cs.append(cc)
    c = np.concatenate(cs).astype(np.int32)
    return s, c
